# revision 12
# baseline (speedup 1.0000x reference)
"""Trainium2 Bass kernel for nn_AssociatorLoss.

Reference computation (B=32, N=32), a = cayley_cube (B,N,N,N), rows a[b,i,j,:]
are probability distributions:

    one[b,i,j,k,l] = sum_m a[b,i,m,l] * a[b,j,k,m]
    two[b,i,j,k,l] = sum_m a[b,m,k,l] * a[b,i,j,m]
    kl = sum(two * (log(two) - log(one))) / B

Strategy (data-parallel over b, 4 batch elements per core, no collectives —
the 8 per-core partial sums are combined on the host):

Per batch element, with x,y,z the three trailing axes of a[b]:
  A  = a[b] in SBUF as [x, (y,z)]        (natural, 32 partitions x 1024)
  AT = 32x32-block transpose of A  -> [z, (y,x)]
  AY = block transpose of A viewed with free dims swapped -> [y, (z,x)]

Matmuls (K = m = 32, bf16, PE):
  two  chunk c (i in [4c,4c+4)):  out[p=(i,j), f=(k,l)] :
       lhsT[m,(i,j)] = AT viewed [z,x,y][:, 4c:4c+4, :],  rhs[m,(k,l)] = A
  one  chunk c:                   out[p=(i,l), f=(k,j)] :
       lhsT[m,(i,l)] = AY viewed [y,x,z][:, 4c:4c+4, :],  rhs[m,(k,j)] = AT

  ("one" comes out with free index (k,j) so that the 32x32-block transpose of
   the "two" chunk — which maps [p=(i,j),f=(k,l)] -> [p=(i,l),f=(k,j)] —
   aligns elementwise with it.)

Elementwise/reduction per chunk:
  ACT:    LT = Ln(two_psum) -> bf16, LO = Ln(one_psum) -> bf16
  DVE:    twot = block-transpose(two_psum);  ttr: P = two_psum*LT, rowsum -> accP
  GPSIMD: stt:  P2 = twot*LO, rowsum -> accM
KL = (sum(accP) - sum(accM)) / B, finished on host in float64.
"""

import sys

for _p in ("/opt/trn_rl_repo",):
    if _p not in sys.path:
        sys.path.insert(0, _p)

import numpy as np

import concourse.bacc as bacc
import concourse.mybir as mybir
import concourse.tile as tile
from concourse.bass_utils import run_bass_kernel_spmd

B, N = 32, 32
N_CORES = 8
B_LOCAL = B // N_CORES  # 4
NCHUNK = (N * N) // 128  # 8 chunks of 128 rows per batch element
F32 = mybir.dt.float32
BF16 = mybir.dt.bfloat16
POOL_COLS = 768


def build(b_local=B_LOCAL, mm_dtype=BF16, log_dtype=F32, reps=1):
    nc = bacc.Bacc(None, target_bir_lowering=False)
    ncols = b_local * NCHUNK
    a_ext = nc.declare_dram_parameter("cayley_cube", [b_local, N, N, N], F32, isOutput=False)
    out_ext = nc.declare_dram_parameter("out", [128, 3 * ncols], F32, isOutput=True)
    av = a_ext.rearrange("b x y z -> b x (y z)")

    mult = mybir.AluOpType.mult
    add = mybir.AluOpType.add

    with tile.TileContext(nc) as tc:
        with (
            tc.tile_pool(name="apool", bufs=2) as apool,
            tc.tile_pool(name="spool", bufs=3) as spool,
            tc.tile_pool(name="scratch", bufs=1) as scratch,
            tc.tile_pool(name="acc", bufs=1) as accpool,
            tc.tile_pool(name="psumT", bufs=2, space="PSUM") as psumT,
            tc.tile_pool(name="psumO", bufs=2, space="PSUM") as psumO,
        ):
            accP = accpool.tile([128, ncols], F32)
            accM = accpool.tile([128, ncols], F32)
            accMp = accpool.tile([1, ncols], F32)
            p1 = scratch.tile([128, 1024], log_dtype)
            p2 = scratch.tile([128, POOL_COLS], log_dtype)
            p2b = scratch.tile([128, 1024 - POOL_COLS], log_dtype)

            for _rep in range(reps):
              for b in range(b_local):
                a32 = apool.tile([N, 1024], F32, tag="a32")
                nc.sync.dma_start(out=a32[:], in_=av[b])
                ab = apool.tile([N, 1024], mm_dtype, tag="ab")
                nc.vector.tensor_copy(ab[:], a32[:])
                # at[z, y*32+x] = a[x,y,z]  (O-matmul rhs: n = k*32+j)
                at = apool.tile([N, 1024], mm_dtype, tag="at")
                nc.vector.transpose(at[:], ab[:])
                # at2[z, x*32+y] = a[x,y,z]  (T-matmul stationary operand:
                # contiguous 128-col slices enumerate (i-group, j))
                at2 = apool.tile([N, 1024], mm_dtype, tag="at2")
                nc.gpsimd.tensor_copy(
                    at2[:].rearrange("p (x y) -> p y x", x=N, y=N),
                    at[:].rearrange("p (y x) -> p y x", y=N, x=N),
                )
                # ay2[y, x*32+z] = a[x,y,z]  (O-matmul stationary operand)
                ay2 = apool.tile([N, 1024], mm_dtype, tag="ay2")
                nc.vector.transpose(ay2[:], at2[:])

                for c in range(NCHUNK):
                    col = b * NCHUNK + c
                    tp = psumT.tile([128, 1024], F32, tag="tp")
                    op = psumO.tile([128, 1024], F32, tag="op")
                    ms = slice(128 * c, 128 * (c + 1))
                    for h in range(2):
                        cs = slice(512 * h, 512 * (h + 1))
                        nc.tensor.matmul(tp[:, cs], at2[:, ms], ab[:, cs],
                                         start=True, stop=True)
                        nc.tensor.matmul(op[:, cs], ay2[:, ms], at[:, cs],
                                         start=True, stop=True)

                    lt = spool.tile([128, 1024], log_dtype, tag="lt")
                    nc.scalar.activation(lt[:], tp[:], mybir.ActivationFunctionType.Ln)
                    lo = spool.tile([128, 1024], log_dtype, tag="lo")
                    nc.scalar.activation(lo[:], op[:], mybir.ActivationFunctionType.Ln)

                    twot = spool.tile([128, 1024], F32, tag="twot")
                    nc.vector.transpose(twot[:], tp[:])

                    nc.vector.scalar_tensor_tensor(
                        out=p1[:], in0=tp[:], scalar=1.0, in1=lt[:],
                        op0=mult, op1=mult, accum_out=accP[:, col:col + 1],
                    )
                    # second dot (aligned pair, all-SBUF): Pool takes the
                    # first POOL_COLS columns, DVE the rest.
                    nc.gpsimd.tensor_tensor(
                        out=p2[:], in0=twot[:, 0:POOL_COLS],
                        in1=lo[:, 0:POOL_COLS], op=mult,
                    )
                    nc.gpsimd.tensor_reduce(
                        out=accMp[:, col:col + 1], in_=p2[:],
                        axis=mybir.AxisListType.XYZWC, op=add,
                    )
                    nc.vector.scalar_tensor_tensor(
                        out=p2b[:], in0=twot[:, POOL_COLS:1024], scalar=1.0,
                        in1=lo[:, POOL_COLS:1024],
                        op0=mult, op1=mult, accum_out=accM[:, col:col + 1],
                    )

            nc.sync.dma_start(out=out_ext[:, 0:ncols], in_=accP[:])
            nc.sync.dma_start(out=out_ext[:, ncols:2 * ncols], in_=accM[:])
            nc.sync.dma_start(out=out_ext[0:1, 2 * ncols:3 * ncols], in_=accMp[:])

    nc.compile()
    return nc


def kernel(cayley_cube: np.ndarray) -> np.ndarray:
    assert cayley_cube.shape == (B, N, N, N)
    nc = build()
    shards = cayley_cube.reshape(N_CORES, B_LOCAL, N, N, N)
    in_maps = [
        {"cayley_cube": np.ascontiguousarray(shards[i])} for i in range(N_CORES)
    ]
    res = run_bass_kernel_spmd(nc, in_maps, core_ids=list(range(N_CORES)))
    ncols = B_LOCAL * NCHUNK
    tot = np.float64(0.0)
    for r in res.results:
        acc = r["out"]
        tot += acc[:, :ncols].sum(dtype=np.float64)
        tot -= acc[:, ncols:2 * ncols].sum(dtype=np.float64)
        tot -= acc[0, 2 * ncols:].sum(dtype=np.float64)
    return np.float32(tot / B)


if __name__ == "__main__":
    rng = np.random.default_rng(0)
    raw = rng.uniform(0.05, 1.0, size=(B, N, N, N)).astype(np.float32)
    a = raw / raw.sum(axis=-1, keepdims=True)
    print(kernel(a))


# revision 14
# speedup vs baseline: 2.4005x; 2.4005x over previous
"""Trainium2 Bass kernel for nn_AssociatorLoss.

Reference computation (B=32, N=32), a = cayley_cube (B,N,N,N), rows a[b,i,j,:]
are probability distributions:

    one[b,i,j,k,l] = sum_m a[b,i,m,l] * a[b,j,k,m]
    two[b,i,j,k,l] = sum_m a[b,m,k,l] * a[b,i,j,m]
    kl = sum(two * (log(two) - log(one))) / B

Strategy (data-parallel over b, 4 batch elements per core, no collectives —
the 8 per-core partial sums are combined on the host):

Per batch element, with x,y,z the three trailing axes of a[b]:
  A  = a[b] in SBUF as [x, (y,z)]        (natural, 32 partitions x 1024)
  AT = 32x32-block transpose of A  -> [z, (y,x)]
  AY = block transpose of A viewed with free dims swapped -> [y, (z,x)]

Matmuls (K = m = 32, bf16, PE):
  two  chunk c (i in [4c,4c+4)):  out[p=(i,j), f=(k,l)] :
       lhsT[m,(i,j)] = AT viewed [z,x,y][:, 4c:4c+4, :],  rhs[m,(k,l)] = A
  one  chunk c:                   out[p=(i,l), f=(k,j)] :
       lhsT[m,(i,l)] = AY viewed [y,x,z][:, 4c:4c+4, :],  rhs[m,(k,j)] = AT

  ("one" comes out with free index (k,j) so that the 32x32-block transpose of
   the "two" chunk — which maps [p=(i,j),f=(k,l)] -> [p=(i,l),f=(k,j)] —
   aligns elementwise with it.)

Elementwise/reduction per chunk:
  ACT:    LT = Ln(two_psum) -> bf16, LO = Ln(one_psum) -> bf16
  DVE:    twot = block-transpose(two_psum);  ttr: P = two_psum*LT, rowsum -> accP
  GPSIMD: stt:  P2 = twot*LO, rowsum -> accM
KL = (sum(accP) - sum(accM)) / B, finished on host in float64.
"""

import sys

for _p in ("/opt/trn_rl_repo",):
    if _p not in sys.path:
        sys.path.insert(0, _p)

import numpy as np

import concourse.bacc as bacc
import concourse.mybir as mybir
import concourse.tile as tile
from concourse.bass_utils import run_bass_kernel_spmd

B, N = 32, 32
N_CORES = 8
B_LOCAL = B // N_CORES  # 4
NCHUNK = (N * N) // 128  # 8 chunks of 128 rows per batch element
F32 = mybir.dt.float32
BF16 = mybir.dt.bfloat16
POOL_COLS = 768


def build(b_local=B_LOCAL, mm_dtype=BF16, log_dtype=F32, reps=1, pool_cols=POOL_COLS, pool_reduce=True, skip=()):
    nc = bacc.Bacc(None, target_bir_lowering=False)
    ncols = b_local * NCHUNK
    a_ext = nc.declare_dram_parameter("cayley_cube", [b_local, N, N, N], F32, isOutput=False)
    out_ext = nc.declare_dram_parameter("out", [128, 3 * ncols], F32, isOutput=True)
    av = a_ext.rearrange("b x y z -> b x (y z)")

    mult = mybir.AluOpType.mult
    add = mybir.AluOpType.add

    with tile.TileContext(nc) as tc:
        with (
            tc.tile_pool(name="apool", bufs=2) as apool,
            tc.tile_pool(name="spool", bufs=3) as spool,
            tc.tile_pool(name="scratch", bufs=1) as scratch,
            tc.tile_pool(name="acc", bufs=1) as accpool,
            tc.tile_pool(name="psumT", bufs=2, space="PSUM") as psumT,
            tc.tile_pool(name="psumO", bufs=2, space="PSUM") as psumO,
        ):
            accP = accpool.tile([128, ncols], F32)
            accM = accpool.tile([128, ncols], F32)
            accMp = accpool.tile([1, ncols], F32)
            p1 = scratch.tile([128, 1024], log_dtype)
            p2 = scratch.tile([128, max(pool_cols, 1)], log_dtype)
            p2b = scratch.tile([128, max(1024 - pool_cols, 1)], log_dtype)

            for _rep in range(reps):
              for b in range(b_local):
                a32 = apool.tile([N, 1024], F32, tag="a32")
                nc.sync.dma_start(out=a32[:], in_=av[b])
                ab = apool.tile([N, 1024], mm_dtype, tag="ab")
                nc.vector.tensor_copy(ab[:], a32[:])
                # at[z, y*32+x] = a[x,y,z]  (O-matmul rhs: n = k*32+j)
                at = apool.tile([N, 1024], mm_dtype, tag="at")
                nc.vector.transpose(at[:], ab[:])
                # at2[z, x*32+y] = a[x,y,z]  (T-matmul stationary operand:
                # contiguous 128-col slices enumerate (i-group, j))
                at2 = apool.tile([N, 1024], mm_dtype, tag="at2")
                nc.gpsimd.tensor_copy(
                    at2[:].rearrange("p (x y) -> p y x", x=N, y=N),
                    at[:].rearrange("p (y x) -> p y x", y=N, x=N),
                )
                # ay2[y, x*32+z] = a[x,y,z]  (O-matmul stationary operand)
                ay2 = apool.tile([N, 1024], mm_dtype, tag="ay2")
                nc.vector.transpose(ay2[:], at2[:])

                for c in range(NCHUNK):
                    col = b * NCHUNK + c
                    tp = psumT.tile([128, 1024], F32, tag="tp")
                    op = psumO.tile([128, 1024], F32, tag="op")
                    ms = slice(128 * c, 128 * (c + 1))
                    for h in range(2):
                        cs = slice(512 * h, 512 * (h + 1))
                        nc.tensor.matmul(tp[:, cs], at2[:, ms], ab[:, cs],
                                         start=True, stop=True)
                        nc.tensor.matmul(op[:, cs], ay2[:, ms], at[:, cs],
                                         start=True, stop=True)

                    lt = spool.tile([128, 1024], log_dtype, tag="lt")
                    lo = spool.tile([128, 1024], log_dtype, tag="lo")
                    if "act" not in skip:
                        nc.scalar.activation(lt[:], tp[:], mybir.ActivationFunctionType.Ln)
                        nc.scalar.activation(lo[:], op[:], mybir.ActivationFunctionType.Ln)

                    twot = spool.tile([128, 1024], F32, tag="twot")
                    if "transpose" not in skip:
                        nc.vector.transpose(twot[:], tp[:])

                    if "dot1" not in skip:
                        nc.vector.scalar_tensor_tensor(
                            out=p1[:], in0=tp[:], scalar=1.0, in1=lt[:],
                            op0=mult, op1=mult, accum_out=accP[:, col:col + 1],
                        )
                    # second dot (aligned pair, all-SBUF): Pool takes the
                    # first pool_cols columns, DVE the rest.
                    if pool_cols > 0 and "pool" not in skip:
                        nc.gpsimd.tensor_tensor(
                            out=p2[:], in0=twot[:, 0:pool_cols],
                            in1=lo[:, 0:pool_cols], op=mult,
                        )
                        if pool_reduce:
                            nc.gpsimd.tensor_reduce(
                                out=accMp[:, col:col + 1], in_=p2[:],
                                axis=mybir.AxisListType.XYZWC, op=add,
                            )
                    if pool_cols < 1024 and "dot2" not in skip:
                        nc.vector.scalar_tensor_tensor(
                            out=p2b[:], in0=twot[:, pool_cols:1024], scalar=1.0,
                            in1=lo[:, pool_cols:1024],
                            op0=mult, op1=mult, accum_out=accM[:, col:col + 1],
                        )

            if "dot1" not in skip:
                nc.sync.dma_start(out=out_ext[:, 0:ncols], in_=accP[:])
            if pool_cols < 1024 and "dot2" not in skip:
                nc.sync.dma_start(out=out_ext[:, ncols:2 * ncols], in_=accM[:])
            if pool_cols > 0 and pool_reduce and "pool" not in skip:
                nc.sync.dma_start(out=out_ext[0:1, 2 * ncols:3 * ncols], in_=accMp[:])

    nc.compile()
    return nc


def kernel(cayley_cube: np.ndarray) -> np.ndarray:
    assert cayley_cube.shape == (B, N, N, N)
    nc = build()
    shards = cayley_cube.reshape(N_CORES, B_LOCAL, N, N, N)
    in_maps = [
        {"cayley_cube": np.ascontiguousarray(shards[i])} for i in range(N_CORES)
    ]
    res = run_bass_kernel_spmd(nc, in_maps, core_ids=list(range(N_CORES)))
    ncols = B_LOCAL * NCHUNK
    tot = np.float64(0.0)
    for r in res.results:
        acc = r["out"]
        tot += acc[:, :ncols].sum(dtype=np.float64)
        tot -= acc[:, ncols:2 * ncols].sum(dtype=np.float64)
        tot -= acc[0, 2 * ncols:].sum(dtype=np.float64)
    return np.float32(tot / B)


if __name__ == "__main__":
    rng = np.random.default_rng(0)
    raw = rng.uniform(0.05, 1.0, size=(B, N, N, N)).astype(np.float32)
    a = raw / raw.sum(axis=-1, keepdims=True)
    print(kernel(a))


# revision 15
# speedup vs baseline: 294.1529x; 122.5360x over previous
"""Trainium2 Bass kernel for nn_AssociatorLoss.

Reference computation (B=32, N=32), a = cayley_cube (B,N,N,N), rows a[b,i,j,:]
are probability distributions:

    one[b,i,j,k,l] = sum_m a[b,i,m,l] * a[b,j,k,m]
    two[b,i,j,k,l] = sum_m a[b,m,k,l] * a[b,i,j,m]
    kl = sum(two * (log(two) - log(one))) / B

Strategy (data-parallel over b, 4 batch elements per core, no collectives —
the 8 per-core partial sums are combined on the host):

Per batch element, with x,y,z the three trailing axes of a[b]:
  A  = a[b] in SBUF as [x, (y,z)]        (natural, 32 partitions x 1024)
  AT = 32x32-block transpose of A  -> [z, (y,x)]
  AY = block transpose of A viewed with free dims swapped -> [y, (z,x)]

Matmuls (K = m = 32, bf16, PE):
  two  chunk c (i in [4c,4c+4)):  out[p=(i,j), f=(k,l)] :
       lhsT[m,(i,j)] = AT viewed [z,x,y][:, 4c:4c+4, :],  rhs[m,(k,l)] = A
  one  chunk c:                   out[p=(i,l), f=(k,j)] :
       lhsT[m,(i,l)] = AY viewed [y,x,z][:, 4c:4c+4, :],  rhs[m,(k,j)] = AT

  ("one" comes out with free index (k,j) so that the 32x32-block transpose of
   the "two" chunk — which maps [p=(i,j),f=(k,l)] -> [p=(i,l),f=(k,j)] —
   aligns elementwise with it.)

Elementwise/reduction per chunk:
  ACT:    LT = Ln(two_psum) -> bf16, LO = Ln(one_psum) -> bf16
  DVE:    twot = block-transpose(two_psum);  ttr: P = two_psum*LT, rowsum -> accP
  GPSIMD: stt:  P2 = twot*LO, rowsum -> accM
KL = (sum(accP) - sum(accM)) / B, finished on host in float64.
"""

import sys

for _p in ("/opt/trn_rl_repo",):
    if _p not in sys.path:
        sys.path.insert(0, _p)

import numpy as np

import concourse.bacc as bacc
import concourse.mybir as mybir
import concourse.tile as tile
from concourse.bass_utils import run_bass_kernel_spmd

B, N = 32, 32
N_CORES = 8
B_LOCAL = B // N_CORES  # 4
NCHUNK = (N * N) // 128  # 8 chunks of 128 rows per batch element
F32 = mybir.dt.float32
BF16 = mybir.dt.bfloat16
POOL_COLS = 768


def build(b_local=B_LOCAL, mm_dtype=BF16, log_dtype=F32, reps=1, pool_cols=POOL_COLS, pool_reduce=True, skip=(), loop_reps=0):
    nc = bacc.Bacc(None, target_bir_lowering=False)
    ncols = b_local * NCHUNK
    a_ext = nc.declare_dram_parameter("cayley_cube", [b_local, N, N, N], F32, isOutput=False)
    out_ext = nc.declare_dram_parameter("out", [128, 3 * ncols], F32, isOutput=True)
    av = a_ext.rearrange("b x y z -> b x (y z)")

    mult = mybir.AluOpType.mult
    add = mybir.AluOpType.add

    with tile.TileContext(nc) as tc:
        with (
            tc.tile_pool(name="apool", bufs=2) as apool,
            tc.tile_pool(name="spool", bufs=3) as spool,
            tc.tile_pool(name="scratch", bufs=1) as scratch,
            tc.tile_pool(name="acc", bufs=1) as accpool,
            tc.tile_pool(name="psumT", bufs=2, space="PSUM") as psumT,
            tc.tile_pool(name="psumO", bufs=2, space="PSUM") as psumO,
        ):
            accP = accpool.tile([128, ncols], F32)
            accM = accpool.tile([128, ncols], F32)
            accMp = accpool.tile([1, ncols], F32)
            p1 = scratch.tile([128, 1024], log_dtype)
            p2 = scratch.tile([128, max(pool_cols, 1)], log_dtype)
            p2b = scratch.tile([128, max(1024 - pool_cols, 1)], log_dtype)

            import contextlib
            loop_ctx = tc.For_i(0, loop_reps, 1) if loop_reps else contextlib.nullcontext()
            with loop_ctx:
             for _rep in range(reps):
              for b in range(b_local):
                a32 = apool.tile([N, 1024], F32, tag="a32")
                nc.sync.dma_start(out=a32[:], in_=av[b])
                ab = apool.tile([N, 1024], mm_dtype, tag="ab")
                nc.vector.tensor_copy(ab[:], a32[:])
                # at[z, y*32+x] = a[x,y,z]  (O-matmul rhs: n = k*32+j)
                at = apool.tile([N, 1024], mm_dtype, tag="at")
                nc.vector.transpose(at[:], ab[:])
                # at2[z, x*32+y] = a[x,y,z]  (T-matmul stationary operand:
                # contiguous 128-col slices enumerate (i-group, j))
                at2 = apool.tile([N, 1024], mm_dtype, tag="at2")
                nc.gpsimd.tensor_copy(
                    at2[:].rearrange("p (x y) -> p y x", x=N, y=N),
                    at[:].rearrange("p (y x) -> p y x", y=N, x=N),
                )
                # ay2[y, x*32+z] = a[x,y,z]  (O-matmul stationary operand)
                ay2 = apool.tile([N, 1024], mm_dtype, tag="ay2")
                nc.vector.transpose(ay2[:], at2[:])

                for c in range(NCHUNK):
                    col = b * NCHUNK + c
                    tp = psumT.tile([128, 1024], F32, tag="tp")
                    op = psumO.tile([128, 1024], F32, tag="op")
                    ms = slice(128 * c, 128 * (c + 1))
                    for h in range(2):
                        cs = slice(512 * h, 512 * (h + 1))
                        nc.tensor.matmul(tp[:, cs], at2[:, ms], ab[:, cs],
                                         start=True, stop=True)
                        nc.tensor.matmul(op[:, cs], ay2[:, ms], at[:, cs],
                                         start=True, stop=True)

                    lt = spool.tile([128, 1024], log_dtype, tag="lt")
                    lo = spool.tile([128, 1024], log_dtype, tag="lo")
                    if "act" not in skip:
                        nc.scalar.activation(lt[:], tp[:], mybir.ActivationFunctionType.Ln)
                        nc.scalar.activation(lo[:], op[:], mybir.ActivationFunctionType.Ln)

                    twot = spool.tile([128, 1024], F32, tag="twot")
                    if "transpose" not in skip:
                        nc.vector.transpose(twot[:], tp[:])

                    if "dot1" not in skip:
                        nc.vector.scalar_tensor_tensor(
                            out=p1[:], in0=tp[:], scalar=1.0, in1=lt[:],
                            op0=mult, op1=mult, accum_out=accP[:, col:col + 1],
                        )
                    # second dot (aligned pair, all-SBUF): Pool takes the
                    # first pool_cols columns, DVE the rest.
                    if pool_cols > 0 and "pool" not in skip:
                        nc.gpsimd.tensor_tensor(
                            out=p2[:], in0=twot[:, 0:pool_cols],
                            in1=lo[:, 0:pool_cols], op=mult,
                        )
                        if pool_reduce:
                            nc.gpsimd.tensor_reduce(
                                out=accMp[:, col:col + 1], in_=p2[:],
                                axis=mybir.AxisListType.XYZWC, op=add,
                            )
                    if pool_cols < 1024 and "dot2" not in skip:
                        nc.vector.scalar_tensor_tensor(
                            out=p2b[:], in0=twot[:, pool_cols:1024], scalar=1.0,
                            in1=lo[:, pool_cols:1024],
                            op0=mult, op1=mult, accum_out=accM[:, col:col + 1],
                        )

            if "dot1" not in skip:
                nc.sync.dma_start(out=out_ext[:, 0:ncols], in_=accP[:])
            if pool_cols < 1024 and "dot2" not in skip:
                nc.sync.dma_start(out=out_ext[:, ncols:2 * ncols], in_=accM[:])
            if pool_cols > 0 and pool_reduce and "pool" not in skip:
                nc.sync.dma_start(out=out_ext[0:1, 2 * ncols:3 * ncols], in_=accMp[:])

    nc.compile()
    return nc


def kernel(cayley_cube: np.ndarray) -> np.ndarray:
    assert cayley_cube.shape == (B, N, N, N)
    nc = build()
    shards = cayley_cube.reshape(N_CORES, B_LOCAL, N, N, N)
    in_maps = [
        {"cayley_cube": np.ascontiguousarray(shards[i])} for i in range(N_CORES)
    ]
    res = run_bass_kernel_spmd(nc, in_maps, core_ids=list(range(N_CORES)))
    ncols = B_LOCAL * NCHUNK
    tot = np.float64(0.0)
    for r in res.results:
        acc = r["out"]
        tot += acc[:, :ncols].sum(dtype=np.float64)
        tot -= acc[:, ncols:2 * ncols].sum(dtype=np.float64)
        tot -= acc[0, 2 * ncols:].sum(dtype=np.float64)
    return np.float32(tot / B)


if __name__ == "__main__":
    rng = np.random.default_rng(0)
    raw = rng.uniform(0.05, 1.0, size=(B, N, N, N)).astype(np.float32)
    a = raw / raw.sum(axis=-1, keepdims=True)
    print(kernel(a))
